# revision 84
# baseline (speedup 1.0000x reference)
"""Trainium2 Bass kernel for nn_EnhancedFractionalPINO.

Math restructuring (all exact, done host-side in fp32/fp64):
  1. The GL fractional derivative is a causal Toeplitz operator T on the
     globally-flattened signal; T^T Ws1 is precomputed by FFT correlation
     (full 12288 in-batch taps + full previous-image halo -> truncation
     error ~1e-7, vs 512-tap truncation in the original kernel).
  2. Re(fft2) is linear: vec(ReF(x))^T W = vec(x)^T (kron-fold W), folded
     per 64x64 block via C^T M C - S^T M S.  Same for the output ifft2.
  3. reference has NO nonlinearity between Ws2 and Wn1, so the whole
     512->12288->512 sandwich collapses to W23 = Ws2 @ Wn1 (512x512).

Result per batch row b:
  h  = relu(x_b @ W1f + tail4096(x_{b-1}) @ Whx + b1)
  h1 = relu(h @ W23 + b23);  h2 = relu(h1 @ Wn2 + bn2)
  out_b = h2 @ W5f + b5f            (fold of ifft2 . Wn3)

Device: 8-way batch parallel (32 rows/core), weight-stationary matmuls
(moving dim = batch 32), W1f/Whx quantized to fp8e3 (E3M4), everything
else fp16, biases applied via scalar.activation / a K=1 matmul. All
weight traffic streams through double-buffered pools so the (exclusive)
DMA pipe stays saturated; PE runs far below it.
"""

import numpy as np
import ml_dtypes

import concourse.mybir as mybir
import concourse.tile as tile
from concourse import bacc
from concourse.bass_utils import run_bass_kernel_spmd

F32 = mybir.dt.float32
F16 = mybir.dt.float16
F8E3 = mybir.dt.float8e3
AF = mybir.ActivationFunctionType

B, C, H, W = 256, 3, 64, 64
MODES = C * H * W              # 12288
ALPHA = 0.5
NTOT = B * MODES
NCORE = 8
BS = B // NCORE                # 32 batches per core
KT = 96                        # k-tiles for L1 main path (halo is on-PE)
NP1 = 6                        # w1 stream pieces (16 k-tiles each)
KP1 = KT // NP1                # 16 k-tiles per piece
NP5 = 12                       # w5 stream pieces (8 out-chunks each)
OCP = 8                        # out-chunks per w5 piece

E3MAX = 15.5


# ---------------------------------------------------------------- host folds
def _pow2_floor(v):
    return float(2.0 ** np.floor(np.log2(v)))


def _fold(x, Ws1, bs1, Ws2, bs2, Wn1, bn1, Wn2, bn2, Wn3, bn3):
    scale_h = float(np.float64(NTOT - 1) ** ALPHA)

    # GL weights, full halo-image span
    NW = 16384
    j = np.arange(1, NW, dtype=np.float64)
    w = np.concatenate([[1.0], np.cumprod((j - 1.0 - ALPHA) / j)])

    # correlation W1a[s,n] = sum_t w[t-s] Ws1[t,n]; halo at s = i-4096 < 0
    L = 32768
    wf = np.fft.rfft(w.astype(np.float64), L)
    sf = np.fft.rfft(Ws1.astype(np.float64), L, axis=0)
    corr = np.fft.irfft(np.conj(wf)[:, None] * sf, L, axis=0)
    W1a = (corr[:MODES] * scale_h).astype(np.float32)          # [12288,512]
    Wha = (corr[L - 4096:] * scale_h).astype(np.float32)       # [4096,512]

    jk = np.outer(np.arange(64), np.arange(64)).astype(np.float64)
    Cm = np.cos(2 * np.pi * jk / 64).astype(np.float32)
    Sm = np.sin(2 * np.pi * jk / 64).astype(np.float32)

    def fold_front(Wblk):      # [k,64,64,512] -> C^T M C - S^T M S
        M4 = Wblk.transpose(0, 3, 1, 2).reshape(-1, 64, 64)
        F = Cm.T @ M4 @ Cm - Sm.T @ M4 @ Sm
        k = Wblk.shape[0]
        return F.reshape(k, 512, 64, 64).transpose(0, 2, 3, 1)

    W1f = fold_front(W1a.reshape(3, 64, 64, 512)).reshape(MODES, 512)
    Whx = fold_front(Wha.reshape(1, 64, 64, 512)).reshape(4096, 512)

    def fold_back(Wblk):       # [R,3,64,64] -> (C M C^T - S M S^T)/4096
        R = Wblk.shape[0]
        M4 = Wblk.reshape(-1, 64, 64)
        F = (Cm @ M4 @ Cm.T - Sm @ M4 @ Sm.T) / np.float32(4096.0)
        return F.reshape(R, 3, 64, 64)

    W23 = Ws2 @ Wn1                                            # [512,512]
    b23 = bs2 @ Wn1 + bn1
    W5f = fold_back(Wn3.reshape(512, 3, 64, 64)).reshape(512, MODES)
    b5f = fold_back(bn3.reshape(1, 3, 64, 64)).reshape(MODES)

    # ---- activation-scale probes (exact fp32 forward pass)
    xf = x.reshape(B, MODES)
    tail = np.zeros((B, 4096), np.float32)
    tail[1:] = xf[:-1, 2 * 4096:]
    pre1 = xf @ W1f + tail @ Whx + bs1
    h = np.maximum(pre1, 0)
    h1 = np.maximum(h @ W23 + b23, 0)
    h2 = np.maximum(h1 @ Wn2 + bn2, 0)
    out = h2 @ W5f + b5f

    e3 = ml_dtypes.float8_e3m4
    # Re(fft2(real x)) obeys A[j,k] = A[64-j,64-k], so only DFT rows 0..32
    # are independent: fold the conjugate-pair weights together. The device
    # computes A rows 0..32 on PE and contracts against Wm.
    W1a4 = W1a.astype(np.float64).reshape(3, 64, 64, 512)
    midx = (64 - np.arange(64)) % 64
    Wm = np.zeros((3, 33, 64, 512))
    for c in range(3):
        Wm[c, 0] = W1a4[c, 0]
        Wm[c, 32] = W1a4[c, 32]
        for jr in range(1, 32):
            Wm[c, jr] = W1a4[c, jr] + W1a4[c, 64 - jr][midx]
    Wm2 = Wm.reshape(3 * 33 * 64, 512).astype(np.float32)
    # per-column exact scale into e3m4's top binade; unscale absorbed
    # into W23's rows (fp16, harmless). Cap so pre1 stays in fp16 range.
    s1c = (15.4 / np.abs(Wm2).max(axis=0)).astype(np.float32)  # [512]
    s1c = np.minimum(s1c, (20000.0 / np.abs(pre1).max(axis=0)).astype(np.float32))
    wm = np.clip(Wm.transpose(0, 2, 1, 3) * s1c, -E3MAX, E3MAX)  # (3,64,33,512)
    # halo tail rows 56..63 of A_prev = rows 8..1 mirrored in k:
    # whp[m', j'-1, n] = Wtail[(8-j')*64 + (64-m')%64, n]
    Wtail = (corr[L - 512:] * scale_h).astype(np.float32)      # [512, 512]
    whp = np.zeros((64, 8, 512), np.float32)
    for jp in range(1, 9):
        whp[:, jp - 1] = Wtail[(8 - jp) * 64 + midx]
    whp = np.clip(whp * s1c[None, None, :], -E3MAX, E3MAX)

    s2 = _pow2_floor(1024.0 / np.abs(h1).max())
    s4 = _pow2_floor(1024.0 / np.abs(h2).max())
    # W5: per-column pow2 boost lifts small columns off the subnormal floor;
    # bounded so od (fp16 device output) stays < ~30000.
    s5g = _pow2_floor(2048.0 / np.abs(out).max())
    W5b = W5f * np.float32(s5g / s4)
    boost = 2.0 ** np.floor(np.log2(15.4 / np.abs(W5b).max(axis=0)))
    cap = 2.0 ** np.floor(np.log2(30000.0 /
                                  (np.abs(out).max(axis=0) * s5g + 1e-9)))
    boost = np.clip(np.minimum(boost, cap), 1.0, 64.0).astype(np.float32)
    W5q = np.clip(W5b * boost, -E3MAX, E3MAX)
    s5v = (s5g * boost).astype(np.float32)                     # per-feature

    f16 = lambda a: np.ascontiguousarray(a, dtype=np.float16)
    f32c = lambda a: np.ascontiguousarray(a, dtype=np.float32)
    jk32 = np.outer(np.arange(64), np.arange(64)).astype(np.float64)
    Cm16 = np.cos(2 * np.pi * jk32 / 64)
    Sm16 = np.sin(2 * np.pi * jk32 / 64)
    # pack all small fp16 consts into one DMA: [128, 264]
    ck = np.zeros((128, 264), np.float16)
    ck[0:64, 0:64] = Cm16
    ck[0:64, 64:128] = Sm16
    ck[0:64, 128:192] = Cm16
    ck[64:128, 128:192] = -Sm16
    ck[0:8, 192:200] = np.eye(8)
    ck[0:1, 200:232] = 1.0                                     # ones row
    # pack fp32 biases b1/b23/b4 into one DMA: [128, 12]
    shared = {
        "w1": np.ascontiguousarray(wm.astype(e3)),             # (3,64,33,512)
        "wh": np.ascontiguousarray(whp.astype(e3)),            # (64,8,512)
        "ck": ck,
        "w23": f16((W23 * (np.float32(s2) / s1c[:, None]))
                   .reshape(4, 128, 512).transpose(1, 0, 2)),  # (128,4,512)
        "w4": f16((Wn2 * np.float32(s4 / s2)).reshape(4, 128, 512)
                  .transpose(1, 0, 2)),
        "w5": np.ascontiguousarray(
            W5q.reshape(4, 128, 96, 128)[:, :, :88]
            .reshape(4, 128, 11, 1024).transpose(2, 1, 0, 3)
            .astype(e3)),                                      # (11,128,4,1024)
        "w5b": np.ascontiguousarray(
            W5q.reshape(4, 128, 96, 128)[:, :, 88:]
            .reshape(4, 128, 2, 512).transpose(2, 1, 0, 3)
            .astype(e3)),                                      # (2,128,4,512)
        "bk": f32c(np.concatenate(
            [(bs1 * s1c).reshape(4, 128).T, (b23 * s2).reshape(4, 128).T,
             (bn2 * s4).reshape(4, 128).T], axis=1)),          # (128,12)
        "b5": f16((b5f * s5v).reshape(1, MODES)),
    }
    return shared, s5v


# ---------------------------------------------------------------- bass module
_NC_CACHE = None


def _build_nc():
    nc = bacc.Bacc("TRN2", target_bir_lowering=False, debug=False,
                   num_devices=NCORE)

    d_w1 = nc.dram_tensor("w1", (3, 64, 33, 512), F8E3, kind="ExternalInput")
    d_wh = nc.dram_tensor("wh", (64, 8, 512), F8E3, kind="ExternalInput")
    d_ck = nc.dram_tensor("ck", (128, 264), F16, kind="ExternalInput")
    d_bk = nc.dram_tensor("bk", (128, 12), F32, kind="ExternalInput")
    d_himg = nc.dram_tensor("himg", (64, 97, 64), F16, kind="ExternalInput")
    d_w23 = nc.dram_tensor("w23", (128, 4, 512), F16, kind="ExternalInput")
    d_w4 = nc.dram_tensor("w4", (128, 4, 512), F16, kind="ExternalInput")
    d_w5 = nc.dram_tensor("w5", (11, 128, 4, 1024), F8E3,
                          kind="ExternalInput")
    d_w5b = nc.dram_tensor("w5b", (2, 128, 4, 512), F8E3,
                           kind="ExternalInput")
    d_b5 = nc.dram_tensor("b5", (1, MODES), F16, kind="ExternalInput")
    d_out = nc.dram_tensor("out", (12, 128, 8 * BS), F16,
                           kind="ExternalOutput")

    with tile.TileContext(nc) as tc:
        with tc.tile_pool(name="cpool", bufs=1) as cpool, \
             tc.tile_pool(name="w1p", bufs=4) as w1p, \
             tc.tile_pool(name="w5p", bufs=12) as w5p, \
             tc.tile_pool(name="odp", bufs=12) as odp, \
             tc.tile_pool(name="hsp", bufs=6) as hsp, \
             tc.tile_pool(name="ps1p", bufs=1, space="PSUM") as ps1p, \
             tc.tile_pool(name="ps24p", bufs=1, space="PSUM") as ps24p, \
             tc.tile_pool(name="psAp", bufs=2, space="PSUM") as psAp, \
             tc.tile_pool(name="psCp", bufs=1, space="PSUM") as psCp, \
             tc.tile_pool(name="ps5p", bufs=2, space="PSUM") as ps5p:
            w23 = cpool.tile([128, 4, 512], F16, tag="w23")
            w4 = cpool.tile([128, 4, 512], F16, tag="w4")
            ckt = cpool.tile([128, 264], F16, tag="ckt")
            bkt = cpool.tile([128, 12], F32, tag="bkt")
            b5r = cpool.tile([1, MODES], F16, tag="b5r")
            h_sb = cpool.tile([128, 4, BS], F16, tag="h_sb")
            h1_sb = cpool.tile([128, 4, BS], F16, tag="h1_sb")
            h2_sb = cpool.tile([128, 4, BS], F16, tag="h2_sb")
            wh = cpool.tile([64, 8, 512], F8E3, tag="wh")
            himg = cpool.tile([64, 97, 64], F16, tag="himg")
            tvm = cpool.tile([64, 33, 97], F16, tag="tvm")
            cs = ckt[0:64, 0:64]
            ssm = ckt[0:64, 64:128]
            lsm = ckt[0:128, 128:192]
            ones = ckt[0:1, 200:200 + BS]
            b1t = bkt[:, 0:4]
            b23t = bkt[:, 4:8]
            b4t = bkt[:, 8:12]

            # All heavyweight DMAs go on the SP queue in exact serve order.
            ps1 = ps1p.tile([128, 4, BS], F32, tag="ps1")
            w1ts = []
            for pc in range(3):
                w1t = w1p.tile([64, 33, 512], F8E3, tag="w1t")
                if pc == 0:
                    nc.sync.dma_start(himg[:], d_himg[:])
                    nc.sync.dma_start(ckt[:], d_ck[:])
                nc.sync.dma_start(w1t[:], d_w1[pc])
                w1ts.append(w1t)
                if pc == 0:
                    nc.sync.dma_start(bkt[:], d_bk[:])
                elif pc == 1:
                    nc.sync.dma_start(wh[:], d_wh[:])
                    nc.sync.dma_start(w23[:], d_w23[:])
                elif pc == 2:
                    nc.sync.dma_start(w4[:], d_w4[:])
                    nc.sync.dma_start(b5r[:], d_b5[:])

            # ---- DFT on PE: A = C x C^T - S x S^T, rows 0..32 only
            # (A[j,k] = A[64-j,64-k] for real x). 97 images, channel-major
            # (img = c*32 + b, slot 96 = cross-core halo image). Stage 2 uses
            # the CONSTANT [C;-S] as stationary so its output is already in
            # [m, (img, j)] layout -- no transpose stage. Emission is
            # software-pipelined one group ahead so PE never waits a copy.
            groups = [(g * 8, 8) for g in range(12)] + [(96, 1)]
            R8s = [None] * len(groups)

            def emit_stage1(g):
                g0, gsz = groups[g]
                psA = psAp.tile([128, 8, 64], F32, tag="psA")
                for i in range(gsz):
                    # R = [x^T C ; x^T S] stacked on partitions (disjoint
                    # partition halves -> both start=True is safe)
                    nc.tensor.matmul(psA[0:64, i, :], himg[:, g0 + i, :],
                                     cs, start=True, stop=True)
                    nc.tensor.matmul(psA[64:128, i, :], himg[:, g0 + i, :],
                                     ssm, start=True, stop=True)
                R8 = hsp.tile([128, 8, 64], F16, tag="R8")
                nc.vector.tensor_copy(R8[:, 0:gsz, :], psA[:, 0:gsz, :])
                R8s[g] = R8

            def emit_stage2(g):
                g0, gsz = groups[g]
                # A[m-part, (img, j)] = sum_p [C;-S][p, m] R[p, (img, j)]
                psC = psCp.tile([64, 8, 34], F32, tag="psC")
                nc.tensor.matmul(psC[:, 0:gsz, 0:33], lsm,
                                 R8s[g][:, 0:gsz, 0:33],
                                 start=True, stop=True, skip_group_check=True)
                nc.scalar.copy(
                    tvm[:, :, g0:g0 + gsz].rearrange("m j b -> m b j"),
                    psC[:, 0:gsz, 0:33])

            # ---- L1 (emitted per channel, interleaved into the DFT
            # pipeline): channel c's matmuls become eligible as soon as its
            # 4 DFT groups land, filling PE stall gaps between groups
            def emit_l1(c):
                w1t = w1ts[c]
                for jr in range(33):
                    rhs = tvm[:, jr, c * BS:(c + 1) * BS]
                    for oc in range(4):
                        # one start=True per PSUM bank: it marks the whole
                        # 2KB bank pending-zero, so siblings must not re-start
                        nc.tensor.matmul(
                            ps1[:, oc, :],
                            w1t[:, jr, oc * 128:(oc + 1) * 128], rhs,
                            start=(c == 0 and jr == 0 and oc == 0),
                            stop=False, skip_group_check=True)

            emit_stage1(0)
            for g in range(len(groups)):
                if g + 1 < len(groups):
                    emit_stage1(g + 1)
                emit_stage2(g)
                if g in (3, 7, 11):
                    emit_l1(g // 4)
            # halo: tail rows 56..63 of A_prev equal rows 8..1 mirrored in k,
            # already computed. Batch b reads ch2 image b-1 (img 64+b-1);
            # batch 0 reads the cross-core image in slot 96.
            for k in range(8):
                for oc in range(4):
                    nc.tensor.matmul(ps1[:, oc, 1:BS],
                                     wh[:, k, oc * 128:(oc + 1) * 128],
                                     tvm[:, k + 1, 64:64 + BS - 1],
                                     start=False, stop=False,
                                     skip_group_check=True)
            for k in range(8):
                for oc in range(4):
                    nc.tensor.matmul(ps1[:, oc, 0:1],
                                     wh[:, k, oc * 128:(oc + 1) * 128],
                                     tvm[:, k + 1, 96:97],
                                     start=False,
                                     stop=(k == 7 and oc == 3),
                                     skip_group_check=True)
            for oc in range(4):
                nc.scalar.activation(h_sb[:, oc, :], ps1[:, oc, :], AF.Relu,
                                     bias=b1t[:, oc:oc + 1])

            # ---- L2: 512 -> 512 (W23)
            ps2 = ps24p.tile([128, 4, BS], F32, tag="ps2")
            for k in range(4):
                for oc in range(4):
                    nc.tensor.matmul(ps2[:, oc, :],
                                     w23[:, k, oc * 128:(oc + 1) * 128],
                                     h_sb[:, k, :],
                                     start=(k == 0 and oc == 0),
                                     stop=(k == 3 and oc == 3),
                                     skip_group_check=True)
            for oc in range(4):
                nc.scalar.activation(h1_sb[:, oc, :], ps2[:, oc, :], AF.Relu,
                                     bias=b23t[:, oc:oc + 1])

            # ---- L4: 512 -> 512 (Wn2)
            ps4 = ps24p.tile([128, 4, BS], F32, tag="ps4")
            for k in range(4):
                for oc in range(4):
                    nc.tensor.matmul(ps4[:, oc, :],
                                     w4[:, k, oc * 128:(oc + 1) * 128],
                                     h1_sb[:, k, :],
                                     start=(k == 0 and oc == 0),
                                     stop=(k == 3 and oc == 3),
                                     skip_group_check=True)
            for oc in range(4):
                nc.scalar.activation(h2_sb[:, oc, :], ps4[:, oc, :], AF.Relu,
                                     bias=b4t[:, oc:oc + 1])

            # ---- L5: 512 -> 12288, 12 out-pieces of 8 chunks; the final
            # piece's WEIGHTS arrive as two half-DMAs so its first matmuls
            # start one transfer earlier, shortening the end-of-kernel chain
            w5ts = []
            for g in range(11):
                w5t = w5p.tile([128, 4, 1024], F8E3, tag="w5t")
                nc.gpsimd.dma_start(w5t[:], d_w5[g])
                w5ts.append(w5t)
            w5l = w5p.tile([128, 4, 1024], F8E3, tag="w5t")
            nc.gpsimd.dma_start(w5l[:, :, 0:512], d_w5b[0])
            nc.gpsimd.dma_start(w5l[:, :, 512:1024], d_w5b[1])
            w5ts.append(w5l)
            for g in range(12):
                w5t = w5ts[g]
                # full-bank tile (2KB): sharing a bank across pieces would
                # let start=True wipe a sibling's live accumulation
                ps5f = ps5p.tile([128, 16, BS], F32, tag="ps5")
                ps5 = ps5f[:, 0:8, :]
                for j in range(8):
                    for k in range(4):
                        nc.tensor.matmul(ps5[:, j, :],
                                         w5t[:, k, j * 128:(j + 1) * 128],
                                         h2_sb[:, k, :],
                                         start=(k == 0 and j == 0), stop=False,
                                         skip_group_check=True)
                    f0 = (g * 8 + j) * 128
                    nc.tensor.matmul(ps5[:, j, :], b5r[0:1, f0:f0 + 128],
                                     ones, start=False,
                                     stop=(j == 7),
                                     skip_group_check=True)
                od = odp.tile([128, 8 * BS], F16, tag="od")
                nc.vector.tensor_copy(
                    od.rearrange("p (j b) -> p j b", j=8), ps5[:])
                nc.scalar.dma_start(d_out[g], od[:])

    nc.compile()
    return nc


def _get_nc():
    global _NC_CACHE
    if _NC_CACHE is None:
        _NC_CACHE = _build_nc()
    return _NC_CACHE


def _make_in_maps(x, Ws1, bs1, Ws2, bs2, Wn1, bn1, Wn2, bn2, Wn3, bn3):
    f32 = np.float32
    shared, s5 = _fold(np.ascontiguousarray(x, f32),
                       *[np.ascontiguousarray(a, f32) for a in
                         (Ws1, bs1, Ws2, bs2, Wn1, bn1, Wn2, bn2, Wn3, bn3)])
    xa = np.ascontiguousarray(x, f32)
    xi = xa.reshape(B, C, 64, 64).astype(np.float16)
    in_maps = []
    for g in range(NCORE):
        blk = np.zeros((97, 64, 64), np.float16)
        # channel-major: img c*32+b = x[g*32+b, c]; slot 96 = cross halo
        blk[0:96] = xi[g * BS:(g + 1) * BS].transpose(1, 0, 2, 3) \
            .reshape(96, 64, 64)
        if g > 0:
            blk[96] = xi[g * BS - 1, 2]
        himg = np.ascontiguousarray(blk.transpose(1, 0, 2))    # (64,97,64)
        in_maps.append({"himg": himg, **shared})
    return in_maps, s5


def kernel(**inputs):
    nc = _get_nc()
    in_maps, s5 = _make_in_maps(
        inputs["x"], inputs["Ws1"], inputs["bs1"], inputs["Ws2"],
        inputs["bs2"], inputs["Wn1"], inputs["bn1"], inputs["Wn2"],
        inputs["bn2"], inputs["Wn3"], inputs["bn3"])
    res = run_bass_kernel_spmd(nc, in_maps, list(range(NCORE)))
    inv = (np.float32(1.0) / s5).astype(np.float32)            # per-feature
    out = np.empty((B, C, H, W), np.float32)
    for g in range(NCORE):
        od = np.asarray(res.results[g]["out"])                 # (12,128,8*32)
        arr = od.reshape(12, 128, 8, BS).astype(np.float32)
        # feature f = (g5*8 + j)*128 + p ; out[b, f]
        feat = arr.transpose(3, 0, 2, 1).reshape(BS, MODES) * inv
        out[g * BS:(g + 1) * BS] = feat.reshape(BS, C, H, W)
    return out


# revision 85
# speedup vs baseline: 1.0099x; 1.0099x over previous
"""Trainium2 Bass kernel for nn_EnhancedFractionalPINO.

Math restructuring (all exact, done host-side in fp32/fp64):
  1. The GL fractional derivative is a causal Toeplitz operator T on the
     globally-flattened signal; T^T Ws1 is precomputed by FFT correlation
     (full 12288 in-batch taps + full previous-image halo -> truncation
     error ~1e-7, vs 512-tap truncation in the original kernel).
  2. Re(fft2) is linear: vec(ReF(x))^T W = vec(x)^T (kron-fold W), folded
     per 64x64 block via C^T M C - S^T M S.  Same for the output ifft2.
  3. reference has NO nonlinearity between Ws2 and Wn1, so the whole
     512->12288->512 sandwich collapses to W23 = Ws2 @ Wn1 (512x512).

Result per batch row b:
  h  = relu(x_b @ W1f + tail4096(x_{b-1}) @ Whx + b1)
  h1 = relu(h @ W23 + b23);  h2 = relu(h1 @ Wn2 + bn2)
  out_b = h2 @ W5f + b5f            (fold of ifft2 . Wn3)

Device: 8-way batch parallel (32 rows/core), weight-stationary matmuls
(moving dim = batch 32), W1f/Whx quantized to fp8e3 (E3M4), everything
else fp16, biases applied via scalar.activation / a K=1 matmul. All
weight traffic streams through double-buffered pools so the (exclusive)
DMA pipe stays saturated; PE runs far below it.
"""

import numpy as np
import ml_dtypes

import concourse.mybir as mybir
import concourse.tile as tile
from concourse import bacc
from concourse.bass_utils import run_bass_kernel_spmd

F32 = mybir.dt.float32
F16 = mybir.dt.float16
F8E3 = mybir.dt.float8e3
AF = mybir.ActivationFunctionType

B, C, H, W = 256, 3, 64, 64
MODES = C * H * W              # 12288
ALPHA = 0.5
NTOT = B * MODES
NCORE = 8
BS = B // NCORE                # 32 batches per core
KT = 96                        # k-tiles for L1 main path (halo is on-PE)
NP1 = 6                        # w1 stream pieces (16 k-tiles each)
KP1 = KT // NP1                # 16 k-tiles per piece
NP5 = 12                       # w5 stream pieces (8 out-chunks each)
OCP = 8                        # out-chunks per w5 piece

E3MAX = 15.5


# ---------------------------------------------------------------- host folds
def _pow2_floor(v):
    return float(2.0 ** np.floor(np.log2(v)))


def _fold(x, Ws1, bs1, Ws2, bs2, Wn1, bn1, Wn2, bn2, Wn3, bn3):
    scale_h = float(np.float64(NTOT - 1) ** ALPHA)

    # GL weights, full halo-image span
    NW = 16384
    j = np.arange(1, NW, dtype=np.float64)
    w = np.concatenate([[1.0], np.cumprod((j - 1.0 - ALPHA) / j)])

    # correlation W1a[s,n] = sum_t w[t-s] Ws1[t,n]; halo at s = i-4096 < 0
    L = 32768
    wf = np.fft.rfft(w.astype(np.float64), L)
    sf = np.fft.rfft(Ws1.astype(np.float64), L, axis=0)
    corr = np.fft.irfft(np.conj(wf)[:, None] * sf, L, axis=0)
    W1a = (corr[:MODES] * scale_h).astype(np.float32)          # [12288,512]
    Wha = (corr[L - 4096:] * scale_h).astype(np.float32)       # [4096,512]

    jk = np.outer(np.arange(64), np.arange(64)).astype(np.float64)
    Cm = np.cos(2 * np.pi * jk / 64).astype(np.float32)
    Sm = np.sin(2 * np.pi * jk / 64).astype(np.float32)

    def fold_front(Wblk):      # [k,64,64,512] -> C^T M C - S^T M S
        M4 = Wblk.transpose(0, 3, 1, 2).reshape(-1, 64, 64)
        F = Cm.T @ M4 @ Cm - Sm.T @ M4 @ Sm
        k = Wblk.shape[0]
        return F.reshape(k, 512, 64, 64).transpose(0, 2, 3, 1)

    W1f = fold_front(W1a.reshape(3, 64, 64, 512)).reshape(MODES, 512)
    Whx = fold_front(Wha.reshape(1, 64, 64, 512)).reshape(4096, 512)

    def fold_back(Wblk):       # [R,3,64,64] -> (C M C^T - S M S^T)/4096
        R = Wblk.shape[0]
        M4 = Wblk.reshape(-1, 64, 64)
        F = (Cm @ M4 @ Cm.T - Sm @ M4 @ Sm.T) / np.float32(4096.0)
        return F.reshape(R, 3, 64, 64)

    W23 = Ws2 @ Wn1                                            # [512,512]
    b23 = bs2 @ Wn1 + bn1
    W5f = fold_back(Wn3.reshape(512, 3, 64, 64)).reshape(512, MODES)
    b5f = fold_back(bn3.reshape(1, 3, 64, 64)).reshape(MODES)

    # ---- activation-scale probes (exact fp32 forward pass)
    xf = x.reshape(B, MODES)
    tail = np.zeros((B, 4096), np.float32)
    tail[1:] = xf[:-1, 2 * 4096:]
    pre1 = xf @ W1f + tail @ Whx + bs1
    h = np.maximum(pre1, 0)
    h1 = np.maximum(h @ W23 + b23, 0)
    h2 = np.maximum(h1 @ Wn2 + bn2, 0)
    out = h2 @ W5f + b5f

    e3 = ml_dtypes.float8_e3m4
    # Re(fft2(real x)) obeys A[j,k] = A[64-j,64-k], so only DFT rows 0..32
    # are independent: fold the conjugate-pair weights together. The device
    # computes A rows 0..32 on PE and contracts against Wm.
    W1a4 = W1a.astype(np.float64).reshape(3, 64, 64, 512)
    midx = (64 - np.arange(64)) % 64
    Wm = np.zeros((3, 33, 64, 512))
    for c in range(3):
        Wm[c, 0] = W1a4[c, 0]
        Wm[c, 32] = W1a4[c, 32]
        for jr in range(1, 32):
            Wm[c, jr] = W1a4[c, jr] + W1a4[c, 64 - jr][midx]
    Wm2 = Wm.reshape(3 * 33 * 64, 512).astype(np.float32)
    # per-column exact scale into e3m4's top binade; unscale absorbed
    # into W23's rows (fp16, harmless). Cap so pre1 stays in fp16 range.
    s1c = (15.4 / np.abs(Wm2).max(axis=0)).astype(np.float32)  # [512]
    s1c = np.minimum(s1c, (20000.0 / np.abs(pre1).max(axis=0)).astype(np.float32))
    wm = np.clip(Wm.transpose(0, 2, 1, 3) * s1c, -E3MAX, E3MAX)  # (3,64,33,512)
    # halo tail rows 56..63 of A_prev = rows 8..1 mirrored in k:
    # whp[m', j'-1, n] = Wtail[(8-j')*64 + (64-m')%64, n]
    Wtail = (corr[L - 512:] * scale_h).astype(np.float32)      # [512, 512]
    whp = np.zeros((64, 8, 512), np.float32)
    for jp in range(1, 9):
        whp[:, jp - 1] = Wtail[(8 - jp) * 64 + midx]
    whp = np.clip(whp * s1c[None, None, :], -E3MAX, E3MAX)

    s2 = _pow2_floor(1024.0 / np.abs(h1).max())
    s4 = _pow2_floor(1024.0 / np.abs(h2).max())
    # W5: per-column pow2 boost lifts small columns off the subnormal floor;
    # bounded so od (fp16 device output) stays < ~30000.
    s5g = _pow2_floor(2048.0 / np.abs(out).max())
    W5b = W5f * np.float32(s5g / s4)
    boost = 2.0 ** np.floor(np.log2(15.4 / np.abs(W5b).max(axis=0)))
    cap = 2.0 ** np.floor(np.log2(30000.0 /
                                  (np.abs(out).max(axis=0) * s5g + 1e-9)))
    boost = np.clip(np.minimum(boost, cap), 1.0, 64.0).astype(np.float32)
    W5q = np.clip(W5b * boost, -E3MAX, E3MAX)
    s5v = (s5g * boost).astype(np.float32)                     # per-feature

    f16 = lambda a: np.ascontiguousarray(a, dtype=np.float16)
    f32c = lambda a: np.ascontiguousarray(a, dtype=np.float32)
    jk32 = np.outer(np.arange(64), np.arange(64)).astype(np.float64)
    Cm16 = np.cos(2 * np.pi * jk32 / 64)
    Sm16 = np.sin(2 * np.pi * jk32 / 64)
    # pack all small fp16 consts into one DMA: [128, 264]
    ck = np.zeros((128, 264), np.float16)
    ck[0:64, 0:64] = Cm16
    ck[0:64, 64:128] = Sm16
    ck[0:64, 128:192] = Cm16
    ck[64:128, 128:192] = -Sm16
    ck[0:8, 192:200] = np.eye(8)
    ck[0:1, 200:232] = 1.0                                     # ones row
    # pack fp32 biases b1/b23/b4 into one DMA: [128, 12]
    shared = {
        "w1": np.ascontiguousarray(wm.astype(e3)),             # (3,64,33,512)
        "wh": np.ascontiguousarray(whp.astype(e3)),            # (64,8,512)
        "ck": ck,
        "w23": f16((W23 * (np.float32(s2) / s1c[:, None]))
                   .reshape(4, 128, 512).transpose(1, 0, 2)),  # (128,4,512)
        "w4": f16((Wn2 * np.float32(s4 / s2)).reshape(4, 128, 512)
                  .transpose(1, 0, 2)),
        "w5": np.ascontiguousarray(
            W5q.reshape(4, 128, 96, 128)[:, :, :88]
            .reshape(4, 128, 11, 1024).transpose(2, 1, 0, 3)
            .astype(e3)),                                      # (11,128,4,1024)
        "w5b": np.ascontiguousarray(
            W5q.reshape(4, 128, 96, 128)[:, :, 88:]
            .reshape(4, 128, 2, 512).transpose(2, 1, 0, 3)
            .astype(e3)),                                      # (2,128,4,512)
        "bk": f32c(np.concatenate(
            [(bs1 * s1c).reshape(4, 128).T, (b23 * s2).reshape(4, 128).T,
             (bn2 * s4).reshape(4, 128).T], axis=1)),          # (128,12)
        "b5": f16((b5f * s5v).reshape(1, MODES)),
    }
    return shared, s5v


# ---------------------------------------------------------------- bass module
_NC_CACHE = None


def _build_nc():
    nc = bacc.Bacc("TRN2", target_bir_lowering=False, debug=False,
                   num_devices=NCORE)

    d_w1 = nc.dram_tensor("w1", (3, 64, 33, 512), F8E3, kind="ExternalInput")
    d_wh = nc.dram_tensor("wh", (64, 8, 512), F8E3, kind="ExternalInput")
    d_ck = nc.dram_tensor("ck", (128, 264), F16, kind="ExternalInput")
    d_bk = nc.dram_tensor("bk", (128, 12), F32, kind="ExternalInput")
    d_himg = nc.dram_tensor("himg", (64, 97, 64), F16, kind="ExternalInput")
    d_w23 = nc.dram_tensor("w23", (128, 4, 512), F16, kind="ExternalInput")
    d_w4 = nc.dram_tensor("w4", (128, 4, 512), F16, kind="ExternalInput")
    d_w5 = nc.dram_tensor("w5", (11, 128, 4, 1024), F8E3,
                          kind="ExternalInput")
    d_w5b = nc.dram_tensor("w5b", (2, 128, 4, 512), F8E3,
                           kind="ExternalInput")
    d_b5 = nc.dram_tensor("b5", (1, MODES), F16, kind="ExternalInput")
    d_out = nc.dram_tensor("out", (12, 128, 8 * BS), F16,
                           kind="ExternalOutput")

    with tile.TileContext(nc) as tc:
        with tc.tile_pool(name="cpool", bufs=1) as cpool, \
             tc.tile_pool(name="w1p", bufs=4) as w1p, \
             tc.tile_pool(name="w5p", bufs=12) as w5p, \
             tc.tile_pool(name="odp", bufs=12) as odp, \
             tc.tile_pool(name="hsp", bufs=6) as hsp, \
             tc.tile_pool(name="ps1p", bufs=1, space="PSUM") as ps1p, \
             tc.tile_pool(name="ps24p", bufs=1, space="PSUM") as ps24p, \
             tc.tile_pool(name="psAp", bufs=2, space="PSUM") as psAp, \
             tc.tile_pool(name="psCp", bufs=1, space="PSUM") as psCp, \
             tc.tile_pool(name="ps5p", bufs=2, space="PSUM") as ps5p:
            w23 = cpool.tile([128, 4, 512], F16, tag="w23")
            w4 = cpool.tile([128, 4, 512], F16, tag="w4")
            ckt = cpool.tile([128, 264], F16, tag="ckt")
            bkt = cpool.tile([128, 12], F32, tag="bkt")
            b5r = cpool.tile([1, MODES], F16, tag="b5r")
            h_sb = cpool.tile([128, 4, BS], F16, tag="h_sb")
            h1_sb = cpool.tile([128, 4, BS], F16, tag="h1_sb")
            h2_sb = cpool.tile([128, 4, BS], F16, tag="h2_sb")
            wh = cpool.tile([64, 8, 512], F8E3, tag="wh")
            himg = cpool.tile([64, 97, 64], F16, tag="himg")
            tvm = cpool.tile([64, 33, 97], F16, tag="tvm")
            cs = ckt[0:64, 0:64]
            ssm = ckt[0:64, 64:128]
            lsm = ckt[0:128, 128:192]
            ones = ckt[0:1, 200:200 + BS]
            b1t = bkt[:, 0:4]
            b23t = bkt[:, 4:8]
            b4t = bkt[:, 8:12]

            # All heavyweight DMAs go on the SP queue in exact serve order.
            ps1 = ps1p.tile([128, 4, BS], F32, tag="ps1")
            w1ts = []
            for pc in range(3):
                w1t = w1p.tile([64, 33, 512], F8E3, tag="w1t")
                if pc == 0:
                    nc.sync.dma_start(himg[:], d_himg[:])
                    nc.sync.dma_start(ckt[:], d_ck[:])
                nc.sync.dma_start(w1t[:], d_w1[pc])
                w1ts.append(w1t)
                if pc == 0:
                    nc.sync.dma_start(bkt[:], d_bk[:])
                elif pc == 1:
                    nc.sync.dma_start(wh[:], d_wh[:])
                    nc.sync.dma_start(w23[:], d_w23[:])
                elif pc == 2:
                    nc.sync.dma_start(w4[:], d_w4[:])
                    nc.sync.dma_start(b5r[:], d_b5[:])

            # ---- DFT on PE: A = C x C^T - S x S^T, rows 0..32 only
            # (A[j,k] = A[64-j,64-k] for real x). 97 images, channel-major
            # (img = c*32 + b, slot 96 = cross-core halo image). Stage 2 uses
            # the CONSTANT [C;-S] as stationary so its output is already in
            # [m, (img, j)] layout -- no transpose stage. Emission is
            # software-pipelined one group ahead so PE never waits a copy.
            groups = [(g * 8, 8) for g in range(12)] + [(96, 1)]
            R8s = [None] * len(groups)

            def emit_stage1(g):
                g0, gsz = groups[g]
                psA = psAp.tile([128, 8, 64], F32, tag="psA")
                for i in range(gsz):
                    # R = [x^T C ; x^T S] stacked on partitions (disjoint
                    # partition halves -> both start=True is safe)
                    nc.tensor.matmul(psA[0:64, i, :], himg[:, g0 + i, :],
                                     cs, start=True, stop=True)
                    nc.tensor.matmul(psA[64:128, i, :], himg[:, g0 + i, :],
                                     ssm, start=True, stop=True)
                R8 = hsp.tile([128, 8, 64], F16, tag="R8")
                nc.vector.tensor_copy(R8[:, 0:gsz, :], psA[:, 0:gsz, :])
                R8s[g] = R8

            def emit_stage2(g):
                g0, gsz = groups[g]
                # A[m-part, (img, j)] = sum_p [C;-S][p, m] R[p, (img, j)]
                psC = psCp.tile([64, 8, 34], F32, tag="psC")
                nc.tensor.matmul(psC[:, 0:gsz, 0:33], lsm,
                                 R8s[g][:, 0:gsz, 0:33],
                                 start=True, stop=True, skip_group_check=True)
                nc.scalar.copy(
                    tvm[:, :, g0:g0 + gsz].rearrange("m j b -> m b j"),
                    psC[:, 0:gsz, 0:33])

            emit_stage1(0)
            for g in range(len(groups)):
                if g + 1 < len(groups):
                    emit_stage1(g + 1)
                emit_stage2(g)

            # ---- L1: contract A rows 0..32 against the folded weights
            for c in range(3):
                w1t = w1ts[c]
                for jr in range(33):
                    rhs = tvm[:, jr, c * BS:(c + 1) * BS]
                    for oc in range(4):
                        # one start=True per PSUM bank: it marks the whole
                        # 2KB bank pending-zero, so siblings must not re-start
                        nc.tensor.matmul(
                            ps1[:, oc, :],
                            w1t[:, jr, oc * 128:(oc + 1) * 128], rhs,
                            start=(c == 0 and jr == 0 and oc == 0),
                            stop=False, skip_group_check=True)
            # halo: tail rows 56..63 of A_prev equal rows 8..1 mirrored in k,
            # already computed. Batch b reads ch2 image b-1 (img 64+b-1);
            # batch 0 reads the cross-core image in slot 96.
            for k in range(8):
                for oc in range(4):
                    nc.tensor.matmul(ps1[:, oc, 1:BS],
                                     wh[:, k, oc * 128:(oc + 1) * 128],
                                     tvm[:, k + 1, 64:64 + BS - 1],
                                     start=False, stop=False,
                                     skip_group_check=True)
            for k in range(8):
                for oc in range(4):
                    nc.tensor.matmul(ps1[:, oc, 0:1],
                                     wh[:, k, oc * 128:(oc + 1) * 128],
                                     tvm[:, k + 1, 96:97],
                                     start=False,
                                     stop=(k == 7 and oc == 3),
                                     skip_group_check=True)
            for oc in range(4):
                nc.scalar.activation(h_sb[:, oc, :], ps1[:, oc, :], AF.Relu,
                                     bias=b1t[:, oc:oc + 1])

            # ---- L2: 512 -> 512 (W23)
            ps2 = ps24p.tile([128, 4, BS], F32, tag="ps2")
            for k in range(4):
                for oc in range(4):
                    nc.tensor.matmul(ps2[:, oc, :],
                                     w23[:, k, oc * 128:(oc + 1) * 128],
                                     h_sb[:, k, :],
                                     start=(k == 0 and oc == 0),
                                     stop=(k == 3 and oc == 3),
                                     skip_group_check=True)
            for oc in range(4):
                nc.scalar.activation(h1_sb[:, oc, :], ps2[:, oc, :], AF.Relu,
                                     bias=b23t[:, oc:oc + 1])

            # ---- L4: 512 -> 512 (Wn2)
            ps4 = ps24p.tile([128, 4, BS], F32, tag="ps4")
            for k in range(4):
                for oc in range(4):
                    nc.tensor.matmul(ps4[:, oc, :],
                                     w4[:, k, oc * 128:(oc + 1) * 128],
                                     h1_sb[:, k, :],
                                     start=(k == 0 and oc == 0),
                                     stop=(k == 3 and oc == 3),
                                     skip_group_check=True)
            for oc in range(4):
                nc.scalar.activation(h2_sb[:, oc, :], ps4[:, oc, :], AF.Relu,
                                     bias=b4t[:, oc:oc + 1])

            # ---- L5: 512 -> 12288, 12 out-pieces of 8 chunks; the final
            # piece's WEIGHTS arrive as two half-DMAs so its first matmuls
            # start one transfer earlier, shortening the end-of-kernel chain
            w5ts = []
            for g in range(11):
                w5t = w5p.tile([128, 4, 1024], F8E3, tag="w5t")
                nc.gpsimd.dma_start(w5t[:], d_w5[g])
                w5ts.append(w5t)
            w5l = w5p.tile([128, 4, 1024], F8E3, tag="w5t")
            nc.gpsimd.dma_start(w5l[:, :, 0:512], d_w5b[0])
            nc.gpsimd.dma_start(w5l[:, :, 512:1024], d_w5b[1])
            w5ts.append(w5l)
            for g in range(12):
                w5t = w5ts[g]
                # full-bank tile (2KB): sharing a bank across pieces would
                # let start=True wipe a sibling's live accumulation
                ps5f = ps5p.tile([128, 16, BS], F32, tag="ps5")
                ps5 = ps5f[:, 0:8, :]
                for j in range(8):
                    for k in range(4):
                        nc.tensor.matmul(ps5[:, j, :],
                                         w5t[:, k, j * 128:(j + 1) * 128],
                                         h2_sb[:, k, :],
                                         start=(k == 0 and j == 0), stop=False,
                                         skip_group_check=True)
                    f0 = (g * 8 + j) * 128
                    nc.tensor.matmul(ps5[:, j, :], b5r[0:1, f0:f0 + 128],
                                     ones, start=False,
                                     stop=(j == 7),
                                     skip_group_check=True)
                od = odp.tile([128, 8 * BS], F16, tag="od")
                nc.vector.tensor_copy(
                    od.rearrange("p (j b) -> p j b", j=8), ps5[:])
                nc.scalar.dma_start(d_out[g], od[:])

    nc.compile()
    return nc


def _get_nc():
    global _NC_CACHE
    if _NC_CACHE is None:
        _NC_CACHE = _build_nc()
    return _NC_CACHE


def _make_in_maps(x, Ws1, bs1, Ws2, bs2, Wn1, bn1, Wn2, bn2, Wn3, bn3):
    f32 = np.float32
    shared, s5 = _fold(np.ascontiguousarray(x, f32),
                       *[np.ascontiguousarray(a, f32) for a in
                         (Ws1, bs1, Ws2, bs2, Wn1, bn1, Wn2, bn2, Wn3, bn3)])
    xa = np.ascontiguousarray(x, f32)
    xi = xa.reshape(B, C, 64, 64).astype(np.float16)
    in_maps = []
    for g in range(NCORE):
        blk = np.zeros((97, 64, 64), np.float16)
        # channel-major: img c*32+b = x[g*32+b, c]; slot 96 = cross halo
        blk[0:96] = xi[g * BS:(g + 1) * BS].transpose(1, 0, 2, 3) \
            .reshape(96, 64, 64)
        if g > 0:
            blk[96] = xi[g * BS - 1, 2]
        himg = np.ascontiguousarray(blk.transpose(1, 0, 2))    # (64,97,64)
        in_maps.append({"himg": himg, **shared})
    return in_maps, s5


def kernel(**inputs):
    nc = _get_nc()
    in_maps, s5 = _make_in_maps(
        inputs["x"], inputs["Ws1"], inputs["bs1"], inputs["Ws2"],
        inputs["bs2"], inputs["Wn1"], inputs["bn1"], inputs["Wn2"],
        inputs["bn2"], inputs["Wn3"], inputs["bn3"])
    res = run_bass_kernel_spmd(nc, in_maps, list(range(NCORE)))
    inv = (np.float32(1.0) / s5).astype(np.float32)            # per-feature
    out = np.empty((B, C, H, W), np.float32)
    for g in range(NCORE):
        od = np.asarray(res.results[g]["out"])                 # (12,128,8*32)
        arr = od.reshape(12, 128, 8, BS).astype(np.float32)
        # feature f = (g5*8 + j)*128 + p ; out[b, f]
        feat = arr.transpose(3, 0, 2, 1).reshape(BS, MODES) * inv
        out[g * BS:(g + 1) * BS] = feat.reshape(BS, C, H, W)
    return out
